# revision 11
# baseline (speedup 1.0000x reference)
"""Viterbi CRF decode on 8 Trainium2 NeuronCores.

Strategy: data-parallel over batch (32 sequences/core). The device kernel runs
the forward max-plus DP (alpha recurrence, the dominant compute) and streams the
full alpha history back to HBM. The host then does the O(L*B*T) backtrack over
that history (0.03% of the FLOPs) plus the sequence-length freeze handling.

Exactness: the device computes alpha_t[j] = max_i(fp32(alpha_{t-1}[i] +
trans[i,j])) + pot_t[j] with the same fp32 rounding as the jax reference, so the
backtrack argmax decisions (first-index tie-break) match bitwise.

Device layout per step (128 partitions = 4 j-quadrants x 32 sequences):
  vt[(q,b), (jb,i)] = alpha[b,i] + trans[i, 16q+jb]   (broadcast add, 1024/partition)
  m4[(q,b), jb]     = max_i vt                        (free-dim reduce)
  alpha'[b, 16q+jb] = m4[(q,b), jb] + pot             (4 collapse copies + add)
"""

import numpy as np

B, L, T = 256, 1024, 64
NCORES = 8
BC = B // NCORES  # 32 sequences per core
CH = 128          # potentials chunk (steps per DMA)

_cache = {}


def _build_program():
    if "nc" in _cache:
        return _cache["nc"]
    import concourse.bacc as bacc
    import concourse.mybir as mybir
    from concourse.tile import TileContext

    f32 = mybir.dt.float32
    AX = mybir.AxisListType
    OP = mybir.AluOpType

    nc = bacc.Bacc("TRN2", target_bir_lowering=False, debug=False)
    pots_in = nc.dram_tensor("pots", [BC, L, T], f32, kind="ExternalInput").ap()
    potq_in = nc.dram_tensor("potq", [128, L, 16], f32, kind="ExternalInput").ap()
    tsp_in = nc.dram_tensor("tspread", [128, 16, T], f32, kind="ExternalInput").ap()
    hist_out = nc.dram_tensor("ahist", [128, L, 16], f32, kind="ExternalOutput").ap()

    JBD = 16  # full add on DVE (no Pool dependency before the reduce)

    with TileContext(nc) as tc:
        with tc.tile_pool(name="const", bufs=1) as cpool, \
             tc.tile_pool(name="pstream", bufs=2) as ppool, \
             tc.tile_pool(name="work", bufs=3) as wpool, \
             tc.tile_pool(name="big", bufs=1) as bpool:
            tsp = cpool.tile([128, 16, T], f32)
            nc.gpsimd.dma_start(out=tsp[:], in_=tsp_in[:])
            hist = bpool.tile([128, L, 16], f32)   # alpha history (q,b), 64KB/partition
            # alpha replicated, split in column halves so the next add's first
            # half can start while the second half's glue is still running
            arepA = cpool.tile([128, 32], f32)
            arepB = cpool.tile([128, 32], f32)

            nchunks = L // CH
            for c in range(nchunks):
                pq = ppool.tile([128, CH, 16], f32, tag="potq")
                nc.gpsimd.dma_start(out=pq[:], in_=potq_in[:, c * CH:(c + 1) * CH, :])

                if c == 0:
                    pc0 = cpool.tile([BC, T], f32)
                    nc.sync.dma_start(out=pc0[:], in_=pots_in[:, 0, :])
                    nc.vector.tensor_copy(arepA[0:BC, :], pc0[:, 0:32])
                    nc.vector.tensor_copy(arepB[0:BC, :], pc0[:, 32:64])
                    nc.scalar.copy(hist[:, 0, :], pq[:, 0, :])
                    nc.vector.tensor_copy(arepA[BC:2 * BC, :], arepA[0:BC, :])
                    nc.vector.tensor_copy(arepA[2 * BC:4 * BC, :], arepA[0:2 * BC, :])
                    nc.vector.tensor_copy(arepB[BC:2 * BC, :], arepB[0:BC, :])
                    nc.vector.tensor_copy(arepB[2 * BC:4 * BC, :], arepB[0:2 * BC, :])

                t0 = max(c * CH, 1)
                for t in range(t0, (c + 1) * CH):
                    s = t - c * CH
                    # vt[p, jb, i] = alpha[p%32, i] + trans[i, 16*(p//32)+jb]
                    vt = wpool.tile([128, 16, T], f32, tag="vt")
                    nc.vector.tensor_add(
                        vt[:, :, 0:32],
                        arepA[:].unsqueeze(1).broadcast_to([128, 16, 32]),
                        tsp[:, :, 0:32],
                    )
                    nc.vector.tensor_add(
                        vt[:, :, 32:64],
                        arepB[:].unsqueeze(1).broadcast_to([128, 16, 32]),
                        tsp[:, :, 32:64],
                    )
                    m4 = wpool.tile([128, 16], f32, tag="m4")
                    nc.vector.tensor_reduce(m4[:], vt[:], axis=AX.X, op=OP.max)
                    # alpha_t in (q,b) layout
                    nc.vector.tensor_add(m4[:], m4[:], pq[:, s, :])
                    nc.scalar.copy(hist[:, t, :], m4[:])
                    # collapse + broadcast, split by alpha-column half:
                    # A half (j 0:32) on DVE(+Act) so the next add-a can start
                    # early; B half (j 32:64) on GpSimd(+Act) in parallel.
                    nc.vector.tensor_copy(arepA[0:BC, 0:16], m4[0:BC, :])
                    nc.vector.tensor_copy(arepA[0:BC, 16:32], m4[BC:2 * BC, :])
                    nc.gpsimd.tensor_copy(arepB[0:BC, 0:16], m4[2 * BC:3 * BC, :])
                    nc.gpsimd.tensor_copy(arepB[0:BC, 16:32], m4[3 * BC:4 * BC, :])
                    nc.vector.tensor_copy(arepA[BC:2 * BC, :], arepA[0:BC, :])
                    nc.scalar.copy(arepA[2 * BC:3 * BC, :], arepA[0:BC, :])
                    nc.vector.tensor_copy(arepA[3 * BC:4 * BC, :], arepA[0:BC, :])
                    nc.gpsimd.tensor_copy(arepB[BC:2 * BC, :], arepB[0:BC, :])
                    nc.scalar.copy(arepB[2 * BC:3 * BC, :], arepB[0:BC, :])
                    nc.gpsimd.tensor_copy(arepB[3 * BC:4 * BC, :], arepB[0:BC, :])

            for tg in range(4):
                nc.gpsimd.dma_start(
                    out=hist_out[:, 256 * tg:256 * (tg + 1), :],
                    in_=hist[:, 256 * tg:256 * (tg + 1), :],
                )

    nc.compile()
    _cache["nc"] = nc
    return nc


def _make_potq(pots):
    # potq[32q + b, t, jb] = pots[b, t, 16q + jb]
    p = pots.reshape(BC, L, 4, 16)                  # [b, t, q, jb]
    return np.ascontiguousarray(p.transpose(2, 0, 1, 3).reshape(128, L, 16))


def _make_tspread(trans):
    # tsp[32q + b, jb, i] = trans[i, 16q + jb]
    tt = np.ascontiguousarray(trans.T).reshape(4, 16, T)  # [q, jb, i]
    return np.repeat(tt[:, None, :, :], BC, axis=1).reshape(128, 16, T).astype(np.float32)


def kernel(potentials, lengths, transition_params):
    from concourse.bass_utils import run_bass_kernel_spmd

    potentials = np.ascontiguousarray(np.asarray(potentials, dtype=np.float32))
    lengths = np.asarray(lengths, dtype=np.int32)
    trans = np.ascontiguousarray(np.asarray(transition_params, dtype=np.float32))

    nc = _build_program()
    tsp = _make_tspread(trans)
    in_maps = [
        {"pots": potentials[c * BC:(c + 1) * BC],
         "potq": _make_potq(potentials[c * BC:(c + 1) * BC]),
         "tspread": tsp}
        for c in range(NCORES)
    ]
    res = run_bass_kernel_spmd(nc, in_maps, core_ids=list(range(NCORES)))
    # hist[(q,b), t, jb] -> ah[b, t, 64]
    ah = np.concatenate(
        [
            res.results[c]["ahist"]
            .reshape(4, BC, L, 16)
            .transpose(1, 2, 0, 3)
            .reshape(BC, L, T)
            for c in range(NCORES)
        ],
        axis=0,
    )

    # Host backtrack over the device-computed alpha history.
    tags = np.zeros((B, L), dtype=np.int64)
    last = ah[np.arange(B), lengths - 1, :].argmax(axis=1)
    tags[:, L - 1] = last
    lm1 = lengths - 1
    for t in range(L - 2, -1, -1):
        nxt = tags[:, t + 1]
        cand = ah[:, t, :] + trans[:, nxt].T
        tags[:, t] = np.where(t >= lm1, last, cand.argmax(axis=1))
    return tags.astype(np.int32)


# revision 12
# speedup vs baseline: 1.0563x; 1.0563x over previous
"""Viterbi CRF decode on 8 Trainium2 NeuronCores.

Strategy: data-parallel over batch (32 sequences/core). The device kernel runs
the forward max-plus DP (alpha recurrence, the dominant compute) and streams the
full alpha history back to HBM. The host then does the O(L*B*T) backtrack over
that history (0.03% of the FLOPs) plus the sequence-length freeze handling.

Exactness: the device computes alpha_t[j] = max_i(fp32(alpha_{t-1}[i] +
trans[i,j])) + pot_t[j] with the same fp32 rounding as the jax reference, so the
backtrack argmax decisions (first-index tie-break) match bitwise.

Device layout per step (128 partitions = 4 j-quadrants x 32 sequences):
  vt[(q,b), (jb,i)] = alpha[b,i] + trans[i, 16q+jb]   (DVE broadcast add)
  m4[(q,b), jb]     = max_i vt                        (DVE free-dim reduce)
  m4 += potq_t                                        (DVE TT on [128,16])
  hist[:, t, :] = m4                                  (Act, off-chain)
  arep[0:32, 16q+jb] = m4[(q,b), jb]                  (collapse, 2 DVE + 2 Pool)
  arep[32:64]/[64:96]/[96:128] <- arep[0:32]          (bcast: DVE, Act, DVE)
"""

import numpy as np

B, L, T = 256, 1024, 64
NCORES = 8
BC = B // NCORES  # 32 sequences per core
CH = 128          # potentials chunk (steps per DMA)

_cache = {}


def _build_program():
    if "nc" in _cache:
        return _cache["nc"]
    import concourse.bacc as bacc
    import concourse.mybir as mybir
    from concourse.tile import TileContext

    f32 = mybir.dt.float32
    AX = mybir.AxisListType
    OP = mybir.AluOpType

    nc = bacc.Bacc("TRN2", target_bir_lowering=False, debug=False)
    pots_in = nc.dram_tensor("pots", [BC, L, T], f32, kind="ExternalInput").ap()
    potq_in = nc.dram_tensor("potq", [128, L, 16], f32, kind="ExternalInput").ap()
    tsp_in = nc.dram_tensor("tspread", [128, 16, T], f32, kind="ExternalInput").ap()
    hist_out = nc.dram_tensor("ahist", [128, L, 16], f32, kind="ExternalOutput").ap()

    with TileContext(nc) as tc:
        with tc.tile_pool(name="const", bufs=1) as cpool, \
             tc.tile_pool(name="pstream", bufs=2) as ppool, \
             tc.tile_pool(name="work", bufs=3) as wpool, \
             tc.tile_pool(name="big", bufs=1) as bpool:
            tsp = cpool.tile([128, 16, T], f32)
            nc.gpsimd.dma_start(out=tsp[:], in_=tsp_in[:])
            hist = bpool.tile([128, L, 16], f32)   # alpha history (q,b), 64KB/partition
            arep = cpool.tile([128, T], f32)

            nchunks = L // CH
            for c in range(nchunks):
                pq = ppool.tile([128, CH, 16], f32, tag="potq")
                nc.gpsimd.dma_start(out=pq[:], in_=potq_in[:, c * CH:(c + 1) * CH, :])

                if c == 0:
                    pc0 = cpool.tile([BC, T], f32)
                    nc.sync.dma_start(out=pc0[:], in_=pots_in[:, 0, :])
                    nc.vector.tensor_copy(arep[0:BC, :], pc0[:])
                    nc.scalar.copy(hist[:, 0, :], pq[:, 0, :])
                    nc.vector.tensor_copy(arep[BC:2 * BC, :], arep[0:BC, :])
                    nc.vector.tensor_copy(arep[2 * BC:4 * BC, :], arep[0:2 * BC, :])

                t0 = max(c * CH, 1)
                for t in range(t0, (c + 1) * CH):
                    s = t - c * CH
                    # vt[p, jb, i] = alpha[p%32, i] + trans[i, 16*(p//32)+jb]
                    vt = wpool.tile([128, 16, T], f32, tag="vt")
                    nc.vector.tensor_add(
                        vt[:],
                        arep[:].unsqueeze(1).broadcast_to([128, 16, T]),
                        tsp[:],
                    )
                    m4 = wpool.tile([128, 16], f32, tag="m4")
                    nc.vector.tensor_reduce(m4[:], vt[:], axis=AX.X, op=OP.max)
                    # alpha_t in (q,b) layout
                    nc.vector.tensor_add(m4[:], m4[:], pq[:, s, :])
                    nc.scalar.copy(hist[:, t, :], m4[:])
                    # collapse to arep[0:32]
                    nc.vector.tensor_copy(arep[0:BC, 0:16], m4[0:BC, :])
                    nc.gpsimd.tensor_copy(arep[0:BC, 16:32], m4[BC:2 * BC, :])
                    nc.vector.tensor_copy(arep[0:BC, 32:48], m4[2 * BC:3 * BC, :])
                    nc.gpsimd.tensor_copy(arep[0:BC, 48:64], m4[3 * BC:4 * BC, :])
                    # broadcast: DVE, Act (own SBUF port), DVE
                    nc.vector.tensor_copy(arep[BC:2 * BC, :], arep[0:BC, :])
                    nc.scalar.copy(arep[2 * BC:3 * BC, :], arep[0:BC, :])
                    nc.vector.tensor_copy(arep[3 * BC:4 * BC, :], arep[0:BC, :])

            for tg in range(4):
                nc.gpsimd.dma_start(
                    out=hist_out[:, 256 * tg:256 * (tg + 1), :],
                    in_=hist[:, 256 * tg:256 * (tg + 1), :],
                )

    nc.compile()
    _cache["nc"] = nc
    return nc


def _make_potq(pots):
    # potq[32q + b, t, jb] = pots[b, t, 16q + jb]
    p = pots.reshape(BC, L, 4, 16)                  # [b, t, q, jb]
    return np.ascontiguousarray(p.transpose(2, 0, 1, 3).reshape(128, L, 16))


def _make_tspread(trans):
    # tsp[32q + b, jb, i] = trans[i, 16q + jb]
    tt = np.ascontiguousarray(trans.T).reshape(4, 16, T)  # [q, jb, i]
    return np.repeat(tt[:, None, :, :], BC, axis=1).reshape(128, 16, T).astype(np.float32)


def kernel(potentials, lengths, transition_params):
    from concourse.bass_utils import run_bass_kernel_spmd

    potentials = np.ascontiguousarray(np.asarray(potentials, dtype=np.float32))
    lengths = np.asarray(lengths, dtype=np.int32)
    trans = np.ascontiguousarray(np.asarray(transition_params, dtype=np.float32))

    nc = _build_program()
    tsp = _make_tspread(trans)
    in_maps = [
        {"pots": potentials[c * BC:(c + 1) * BC],
         "potq": _make_potq(potentials[c * BC:(c + 1) * BC]),
         "tspread": tsp}
        for c in range(NCORES)
    ]
    res = run_bass_kernel_spmd(nc, in_maps, core_ids=list(range(NCORES)))
    # hist[(q,b), t, jb] -> ah[b, t, 64]
    ah = np.concatenate(
        [
            res.results[c]["ahist"]
            .reshape(4, BC, L, 16)
            .transpose(1, 2, 0, 3)
            .reshape(BC, L, T)
            for c in range(NCORES)
        ],
        axis=0,
    )

    # Host backtrack over the device-computed alpha history.
    tags = np.zeros((B, L), dtype=np.int64)
    last = ah[np.arange(B), lengths - 1, :].argmax(axis=1)
    tags[:, L - 1] = last
    lm1 = lengths - 1
    for t in range(L - 2, -1, -1):
        nxt = tags[:, t + 1]
        cand = ah[:, t, :] + trans[:, nxt].T
        tags[:, t] = np.where(t >= lm1, last, cand.argmax(axis=1))
    return tags.astype(np.int32)


# revision 13
# speedup vs baseline: 1.0680x; 1.0111x over previous
"""Viterbi CRF decode on 8 Trainium2 NeuronCores.

Strategy: data-parallel over batch (32 sequences/core). The device kernel runs
the forward max-plus DP (alpha recurrence, the dominant compute) and streams the
full alpha history back to HBM. The host then does the O(L*B*T) backtrack over
that history (0.03% of the FLOPs) plus the sequence-length freeze handling.

Exactness: the device computes alpha_t[j] = max_i(fp32(alpha_{t-1}[i] +
trans[i,j])) + pot_t[j] with the same fp32 rounding as the jax reference, so the
backtrack argmax decisions (first-index tie-break) match bitwise.

Device layout per step (128 partitions = 4 j-quadrants x 32 sequences):
  vt[(q,b), (jb,i)] = alpha[b,i] + trans[i, 16q+jb]   (DVE broadcast add)
  m4[(q,b), jb]     = max_i vt                        (DVE free-dim reduce)
  m4 += potq_t                                        (DVE TT on [128,16])
  hist[:, t, :] = m4                                  (Act, off-chain)
  arep[0:32, 16q+jb] = m4[(q,b), jb]                  (collapse, 2 DVE + 2 Pool)
  arep[32:64]/[64:96]/[96:128] <- arep[0:32]          (bcast: DVE, Act, DVE)
"""

import numpy as np

B, L, T = 256, 1024, 64
NCORES = 8
BC = B // NCORES  # 32 sequences per core
CH = 128          # potentials chunk (steps per DMA)

_cache = {}


def _build_program():
    if "nc" in _cache:
        return _cache["nc"]
    import concourse.bacc as bacc
    import concourse.mybir as mybir
    from concourse.tile import TileContext

    f32 = mybir.dt.float32
    AX = mybir.AxisListType
    OP = mybir.AluOpType

    nc = bacc.Bacc("TRN2", target_bir_lowering=False, debug=False)
    pots_in = nc.dram_tensor("pots", [BC, L, T], f32, kind="ExternalInput").ap()
    potq_in = nc.dram_tensor("potq", [128, L, 16], f32, kind="ExternalInput").ap()
    tsp_in = nc.dram_tensor("tspread", [128, 16, T], f32, kind="ExternalInput").ap()
    hist_out = nc.dram_tensor("ahist", [128, L, 16], f32, kind="ExternalOutput").ap()

    with TileContext(nc) as tc:
        with tc.tile_pool(name="const", bufs=1) as cpool, \
             tc.tile_pool(name="pstream", bufs=2) as ppool, \
             tc.tile_pool(name="work", bufs=3) as wpool, \
             tc.tile_pool(name="big", bufs=1) as bpool:
            tsp = cpool.tile([128, 16, T], f32)
            nc.gpsimd.dma_start(out=tsp[:], in_=tsp_in[:])
            hist = bpool.tile([128, L, 16], f32)   # alpha history (q,b), 64KB/partition
            arep = cpool.tile([128, T], f32)

            nchunks = L // CH
            for c in range(nchunks):
                pq = ppool.tile([128, CH, 16], f32, tag="potq")
                nc.gpsimd.dma_start(out=pq[:], in_=potq_in[:, c * CH:(c + 1) * CH, :])

                if c == 0:
                    pc0 = cpool.tile([BC, T], f32)
                    nc.sync.dma_start(out=pc0[:], in_=pots_in[:, 0, :])
                    nc.vector.tensor_copy(arep[0:BC, :], pc0[:])
                    nc.scalar.copy(hist[:, 0, :], pq[:, 0, :])
                    nc.vector.tensor_copy(arep[BC:2 * BC, :], arep[0:BC, :])
                    nc.vector.tensor_copy(arep[2 * BC:4 * BC, :], arep[0:2 * BC, :])

                t0 = max(c * CH, 1)
                for t in range(t0, (c + 1) * CH):
                    s = t - c * CH
                    # vt[p, jb, i] = alpha[p%32, i] + trans[i, 16*(p//32)+jb]
                    vt = wpool.tile([128, 16, T], f32, tag="vt")
                    nc.vector.tensor_add(
                        vt[:],
                        arep[:].unsqueeze(1).broadcast_to([128, 16, T]),
                        tsp[:],
                    )
                    m4 = wpool.tile([128, 16], f32, tag="m4")
                    nc.vector.tensor_reduce(m4[:], vt[:], axis=AX.X, op=OP.max)
                    # alpha_t in (q,b) layout
                    nc.vector.tensor_add(m4[:], m4[:], pq[:, s, :])
                    # collapse to arep[0:32]: 2 DVE + 1 GpSimd + 1 Act
                    nc.vector.tensor_copy(arep[0:BC, 0:16], m4[0:BC, :])
                    nc.gpsimd.tensor_copy(arep[0:BC, 16:32], m4[BC:2 * BC, :])
                    nc.vector.tensor_copy(arep[0:BC, 32:48], m4[2 * BC:3 * BC, :])
                    nc.scalar.copy(arep[0:BC, 48:64], m4[3 * BC:4 * BC, :])
                    # broadcast: DVE, Act, GpSimd; hist last on Act (off-chain)
                    nc.vector.tensor_copy(arep[BC:2 * BC, :], arep[0:BC, :])
                    nc.scalar.copy(arep[2 * BC:3 * BC, :], arep[0:BC, :])
                    nc.gpsimd.tensor_copy(arep[3 * BC:4 * BC, :], arep[0:BC, :])
                    nc.scalar.copy(hist[:, t, :], m4[:])

            for tg in range(4):
                nc.gpsimd.dma_start(
                    out=hist_out[:, 256 * tg:256 * (tg + 1), :],
                    in_=hist[:, 256 * tg:256 * (tg + 1), :],
                )

    nc.compile()
    _cache["nc"] = nc
    return nc


def _make_potq(pots):
    # potq[32q + b, t, jb] = pots[b, t, 16q + jb]
    p = pots.reshape(BC, L, 4, 16)                  # [b, t, q, jb]
    return np.ascontiguousarray(p.transpose(2, 0, 1, 3).reshape(128, L, 16))


def _make_tspread(trans):
    # tsp[32q + b, jb, i] = trans[i, 16q + jb]
    tt = np.ascontiguousarray(trans.T).reshape(4, 16, T)  # [q, jb, i]
    return np.repeat(tt[:, None, :, :], BC, axis=1).reshape(128, 16, T).astype(np.float32)


def kernel(potentials, lengths, transition_params):
    from concourse.bass_utils import run_bass_kernel_spmd

    potentials = np.ascontiguousarray(np.asarray(potentials, dtype=np.float32))
    lengths = np.asarray(lengths, dtype=np.int32)
    trans = np.ascontiguousarray(np.asarray(transition_params, dtype=np.float32))

    nc = _build_program()
    tsp = _make_tspread(trans)
    in_maps = [
        {"pots": potentials[c * BC:(c + 1) * BC],
         "potq": _make_potq(potentials[c * BC:(c + 1) * BC]),
         "tspread": tsp}
        for c in range(NCORES)
    ]
    res = run_bass_kernel_spmd(nc, in_maps, core_ids=list(range(NCORES)))
    # hist[(q,b), t, jb] -> ah[b, t, 64]
    ah = np.concatenate(
        [
            res.results[c]["ahist"]
            .reshape(4, BC, L, 16)
            .transpose(1, 2, 0, 3)
            .reshape(BC, L, T)
            for c in range(NCORES)
        ],
        axis=0,
    )

    # Host backtrack over the device-computed alpha history.
    tags = np.zeros((B, L), dtype=np.int64)
    last = ah[np.arange(B), lengths - 1, :].argmax(axis=1)
    tags[:, L - 1] = last
    lm1 = lengths - 1
    for t in range(L - 2, -1, -1):
        nxt = tags[:, t + 1]
        cand = ah[:, t, :] + trans[:, nxt].T
        tags[:, t] = np.where(t >= lm1, last, cand.argmax(axis=1))
    return tags.astype(np.int32)


# revision 14
# speedup vs baseline: 1.0922x; 1.0227x over previous
"""Viterbi CRF decode on 8 Trainium2 NeuronCores.

Strategy: data-parallel over batch (32 sequences/core). The device kernel runs
the forward max-plus DP (alpha recurrence, the dominant compute) and streams the
full alpha history back to HBM. The host then does the O(L*B*T) backtrack over
that history (0.03% of the FLOPs) plus the sequence-length freeze handling.

Exactness: the device computes alpha_t[j] = max_i(fp32(alpha_{t-1}[i] +
trans[i,j])) + pot_t[j] with the same fp32 rounding as the jax reference, so the
backtrack argmax decisions (first-index tie-break) match bitwise.

Device layout per step (128 partitions = 4 j-quadrants x 32 sequences):
  vt[(q,b), (jb,i)] = alpha[b,i] + trans[i, 16q+jb]   (DVE broadcast add)
  m4[(q,b), jb]     = max_i vt                        (DVE free-dim reduce)
  m4 += potq_t                                        (DVE TT on [128,16])
  hist[:, t, :] = m4                                  (Act, off-chain)
  arep[0:32, 16q+jb] = m4[(q,b), jb]                  (collapse, 2 DVE + 2 Pool)
  arep[32:64]/[64:96]/[96:128] <- arep[0:32]          (bcast: DVE, Act, DVE)
"""

import numpy as np

B, L, T = 256, 1024, 64
NCORES = 8
BC = B // NCORES  # 32 sequences per core
CH = 128          # potentials chunk (steps per DMA)

_cache = {}


def _build_program():
    if "nc" in _cache:
        return _cache["nc"]
    import concourse.bacc as bacc
    import concourse.mybir as mybir
    from concourse.tile import TileContext

    f32 = mybir.dt.float32
    AX = mybir.AxisListType
    OP = mybir.AluOpType

    nc = bacc.Bacc("TRN2", target_bir_lowering=False, debug=False)
    pots_in = nc.dram_tensor("pots", [BC, L, T], f32, kind="ExternalInput").ap()
    potq_in = nc.dram_tensor("potq", [128, L, 16], f32, kind="ExternalInput").ap()
    tsp_in = nc.dram_tensor("tspread", [128, 16, T], f32, kind="ExternalInput").ap()
    hist_out = nc.dram_tensor("ahist", [128, L, 16], f32, kind="ExternalOutput").ap()

    with TileContext(nc) as tc:
        with tc.tile_pool(name="const", bufs=1) as cpool, \
             tc.tile_pool(name="pstream", bufs=2) as ppool, \
             tc.tile_pool(name="work", bufs=3) as wpool, \
             tc.tile_pool(name="big", bufs=1) as bpool:
            tsp = cpool.tile([128, 16, T], f32)
            nc.gpsimd.dma_start(out=tsp[:], in_=tsp_in[:])
            hist = bpool.tile([128, L, 16], f32)   # alpha history (q,b), 64KB/partition
            arep = cpool.tile([128, T], f32)

            nchunks = L // CH
            for c in range(nchunks):
                pq = ppool.tile([128, CH, 16], f32, tag="potq")
                nc.gpsimd.dma_start(out=pq[:], in_=potq_in[:, c * CH:(c + 1) * CH, :])

                if c == 0:
                    pc0 = cpool.tile([BC, T], f32)
                    nc.sync.dma_start(out=pc0[:], in_=pots_in[:, 0, :])
                    nc.vector.tensor_copy(arep[0:BC, :], pc0[:])
                    nc.scalar.copy(hist[:, 0, :], pq[:, 0, :])
                    nc.vector.tensor_copy(arep[BC:2 * BC, :], arep[0:BC, :])
                    nc.vector.tensor_copy(arep[2 * BC:4 * BC, :], arep[0:2 * BC, :])

                t0 = max(c * CH, 1)
                for t in range(t0, (c + 1) * CH):
                    s = t - c * CH
                    # vt[p, jb, i] = alpha[p%32, i] + trans[i, 16*(p//32)+jb]
                    vt = wpool.tile([128, 16, T], f32, tag="vt")
                    nc.vector.tensor_add(
                        vt[:],
                        arep[:].unsqueeze(1).broadcast_to([128, 16, T]),
                        tsp[:],
                    )
                    m4 = wpool.tile([128, 16], f32, tag="m4")
                    nc.vector.tensor_reduce(m4[:], vt[:], axis=AX.X, op=OP.max)
                    # alpha_t in (q,b) layout
                    nc.vector.tensor_add(m4[:], m4[:], pq[:, s, :])
                    # collapse to arep[0:32]: 2 DVE + 1 GpSimd + 1 Act
                    nc.vector.tensor_copy(arep[0:BC, 0:16], m4[0:BC, :])
                    nc.gpsimd.tensor_copy(arep[0:BC, 16:32], m4[BC:2 * BC, :])
                    nc.vector.tensor_copy(arep[0:BC, 32:48], m4[2 * BC:3 * BC, :])
                    nc.scalar.copy(arep[0:BC, 48:64], m4[3 * BC:4 * BC, :])
                    # broadcast: DVE, Act, DVE; hist last on Act (off-chain)
                    nc.vector.tensor_copy(arep[BC:2 * BC, :], arep[0:BC, :])
                    nc.scalar.copy(arep[2 * BC:3 * BC, :], arep[0:BC, :])
                    nc.vector.tensor_copy(arep[3 * BC:4 * BC, :], arep[0:BC, :])
                    nc.scalar.copy(hist[:, t, :], m4[:])

            for tg in range(4):
                nc.gpsimd.dma_start(
                    out=hist_out[:, 256 * tg:256 * (tg + 1), :],
                    in_=hist[:, 256 * tg:256 * (tg + 1), :],
                )

    nc.compile()
    _cache["nc"] = nc
    return nc


def _make_potq(pots):
    # potq[32q + b, t, jb] = pots[b, t, 16q + jb]
    p = pots.reshape(BC, L, 4, 16)                  # [b, t, q, jb]
    return np.ascontiguousarray(p.transpose(2, 0, 1, 3).reshape(128, L, 16))


def _make_tspread(trans):
    # tsp[32q + b, jb, i] = trans[i, 16q + jb]
    tt = np.ascontiguousarray(trans.T).reshape(4, 16, T)  # [q, jb, i]
    return np.repeat(tt[:, None, :, :], BC, axis=1).reshape(128, 16, T).astype(np.float32)


def kernel(potentials, lengths, transition_params):
    from concourse.bass_utils import run_bass_kernel_spmd

    potentials = np.ascontiguousarray(np.asarray(potentials, dtype=np.float32))
    lengths = np.asarray(lengths, dtype=np.int32)
    trans = np.ascontiguousarray(np.asarray(transition_params, dtype=np.float32))

    nc = _build_program()
    tsp = _make_tspread(trans)
    in_maps = [
        {"pots": potentials[c * BC:(c + 1) * BC],
         "potq": _make_potq(potentials[c * BC:(c + 1) * BC]),
         "tspread": tsp}
        for c in range(NCORES)
    ]
    res = run_bass_kernel_spmd(nc, in_maps, core_ids=list(range(NCORES)))
    # hist[(q,b), t, jb] -> ah[b, t, 64]
    ah = np.concatenate(
        [
            res.results[c]["ahist"]
            .reshape(4, BC, L, 16)
            .transpose(1, 2, 0, 3)
            .reshape(BC, L, T)
            for c in range(NCORES)
        ],
        axis=0,
    )

    # Host backtrack over the device-computed alpha history.
    tags = np.zeros((B, L), dtype=np.int64)
    last = ah[np.arange(B), lengths - 1, :].argmax(axis=1)
    tags[:, L - 1] = last
    lm1 = lengths - 1
    for t in range(L - 2, -1, -1):
        nxt = tags[:, t + 1]
        cand = ah[:, t, :] + trans[:, nxt].T
        tags[:, t] = np.where(t >= lm1, last, cand.argmax(axis=1))
    return tags.astype(np.int32)
